# revision 35
# baseline (speedup 1.0000x reference)
"""Trainium2 Bass kernel for the KB criterion loss.

Math
----
reference:
    diff[b,i,j] = probs[b,j] - probs[b,i]
    loss = sum_ij mean_b (diff^2 * C[i,j]) / (n_pos + 1e-8),  n_pos = count(C > 0)

Expanding the square removes the [B,N,N] intermediate entirely:

    sum_b (P[b,i] - P[b,j])^2 = S2_i + S2_j - 2*G_ij
        with S2_j = sum_b P[b,j]^2   and   G = P^T P  (Gram matrix)

so   total = sum_ij C_ij * (S2_i + S2_j - 2 G_ij)
     loss  = (total / B) / (n_pos + 1e-8)

Sharding (8 cores)
------------------
Shard C by rows: core k owns rows S_k = [128k, 128k+128). P is replicated.
Each core moves ~0.27MB (fp8 [PT|P] + fp8 C shard) vs 4MB of C with
batch-parallel sharding — fewer bytes also shrinks the cross-core HBM
contention window that makes straggler cores. Inputs are column-rolled by 128k so every core
runs the same program with its own row block mapped to local columns
[0:128). The transposed local P block (pure layout prep, like the roll)
rides in front of P in one fused DMA so S2_i is available as a
per-partition column without an on-device transpose.

Per-core pipeline (engines in parallel; inputs fp8e4m3, psq bf16,
accumulation fp32):
  DVE : psq = P*P (256-col quarters, so the S-matmuls start early)
  PE  : per 512-col PSUM bank (one tile per bank so each bank's
        accumulation group closes independently):
           d = P_Sk^T @ P            (Gram block, contract over b=128)
             + (-1/2 const)^T @ psq  (= -S2_j/2 broadcast, 256-col stops)
  ACT : s2i = rowsum(PT_Sk^2)  (Square + accum)  -> S2_i column
  DVE : (C * -2) * d  summed per partition  -> red[:,0:2]  (fused STT)
  ACT : C * s2i [0:896]  summed per partition -> red[:,2] (Copy, scale AP)
  DVE : C * s2i [896:1024]                    -> red[:,4] (tensor_scalar)
  ACT : Sign(C)  summed per partition         -> red[:,3] (n_pos count)
  SP  : DMA the [128,5] partials out; host does the partition reduce.

The ACT/DVE split of the C*S2_i pass balances the two engines' post-C
work. Host sums the 8 partial [128,5] blocks (the scalar all-reduce)
and finishes the division. fp8 input rounding adds a ~0.4% bias on the
squared differences plus random error that cancels over the 2^20
summed terms — observed rel err 5.5e-3 vs the 2e-2 tolerance.
"""

import ml_dtypes
import numpy as np

import concourse.bass as bass
import concourse.tile as tile
from concourse import mybir
from concourse.bass_utils import run_bass_kernel_spmd

B = 128
N = 1024
NCORES = 8
SH = N // NCORES  # 128 rows of C per core
F32 = mybir.dt.float32
BF16 = mybir.dt.bfloat16
HALF = 512  # PSUM bank width in fp32
BF16NP = ml_dtypes.bfloat16
FP8 = mybir.dt.float8e4
FP8NP = ml_dtypes.float8_e4m3


def build_bass() -> bass.Bass:
    nc = bass.Bass(monotonic_sem_count=0, enable_partition_id=False)
    # Drop the dead const-AP memsets (f32-1.0, bf16-1.0, u8-127) from the
    # preamble: they run on Pool, the straggler of the start barrier. The
    # first one (f32-0.0) stays — the Sign bias reads it.
    _mb = nc.main_func.blocks[0]
    _memsets = [i for i in _mb.instructions if isinstance(i, mybir.InstMemset)]
    for _ins in _memsets[1:]:
        _mb.instructions.remove(_ins)
    # paug = [PT_Sk | P] fused into one DMA: one issue slot instead of
    # two, so the C transfer starts ~0.6us earlier.
    paug_d = nc.dram_tensor("paug_r", [B, SH + N], FP8, kind="ExternalInput")
    c_d = nc.dram_tensor("co_r", [SH, N], FP8, kind="ExternalInput")
    o_d = nc.dram_tensor("out", [SH, 5], F32, kind="ExternalOutput")

    with tile.TileContext(nc) as tc:
        with (
            tc.tile_pool(name="sb", bufs=1) as sb,
            tc.tile_pool(name="ps", bufs=1, space="PSUM") as ps,
        ):
            paug_sb = sb.tile([B, SH + N], FP8)
            pt_sb = paug_sb[:, 0:SH]
            p_sb = paug_sb[:, SH : SH + N]
            c_sb = sb.tile([SH, N], FP8)
            psq = sb.tile([B, N], BF16)
            nh = sb.tile([B, SH], BF16)  # const -1/2, lhsT of the S2_j matmul
            ptsq = sb.tile([SH, B], BF16)  # dead store of the Square pass
            s2i = sb.tile([SH, 1], F32)
            scr = sb.tile([SH, N], BF16)  # dead store of the fused STT
            scra = sb.tile([SH, N], BF16)  # dead store of the C*s2i pass
            scrs = sb.tile([SH, N], BF16)  # dead store of the Sign pass
            red = sb.tile([SH, 5], F32)

            # one PSUM tile per bank: a reader of a PSUM tile waits for the
            # tile's whole accumulation group, so separate tiles let the
            # first STT start as soon as bank0's group closes
            d_ps = [
                ps.tile([B, HALF], F32, name=f"d_ps{h}") for h in range(2)
            ]

            # Constant: the consumer S-matmul pairs DVE-produced psq with
            # this, so it is born on DVE to keep matmul waits cheap.
            nc.vector.memset(nh, -0.5)

            # Loads, all on the SP queue: split queues measured slower (the
            # 16 DMA engines are shared and the queues contend). [PT|P]
            # first (heads the compute path), then C.
            nc.sync.dma_start(out=paug_sb, in_=paug_d[:, :])
            nc.sync.dma_start(out=c_sb, in_=c_d[:, :])

            # psq = P*P in quarters so the S-matmuls can start earlier
            Q = 256
            for q in range(4):
                qs = slice(Q * q, Q * (q + 1))
                nc.vector.tensor_mul(psq[:, qs], p_sb[:, qs], p_sb[:, qs])

            # s2i[i] = sum_b PT[i,b]^2 = S2 for the local rows, as a column
            nc.scalar.activation(
                ptsq, pt_sb, mybir.ActivationFunctionType.Square, accum_out=s2i
            )
            # red[:,2] = sum_j C * S2_i over [0:896] (scale is a
            # per-partition column); the last eighth runs on DVE (red[:,4])
            # to balance the two engines' post-C work
            nc.scalar.activation(
                scra[:, 0:896],
                c_sb[:, 0:896],
                mybir.ActivationFunctionType.Copy,
                scale=s2i[:, 0:1],
                accum_out=red[:, 2:3],
            )

            # d = G - S2_j/2 accumulated per PSUM bank; the S term lands in
            # 256-col quarters so each bank's tail quarter stops sooner
            for h in range(2):
                js = slice(HALF * h, HALF * (h + 1))
                nc.tensor.matmul(
                    d_ps[h][:, :], p_sb[:, 0:SH], p_sb[:, js], start=True, stop=False
                )
                for hq in range(2):
                    qs = slice(Q * hq, Q * (hq + 1))
                    nc.tensor.matmul(
                        d_ps[h][:, qs],
                        nh,
                        psq[:, HALF * h + Q * hq : HALF * h + Q * (hq + 1)],
                        start=False,
                        stop=True,
                    )

            # red[:,h] = sum_j (C * -2) * d  (fused multiply+accumulate)
            for h in range(2):
                js = slice(HALF * h, HALF * (h + 1))
                nc.vector.scalar_tensor_tensor(
                    scr[:, js],
                    c_sb[:, js],
                    -2.0,
                    d_ps[h][:, :],
                    op0=mybir.AluOpType.mult,
                    op1=mybir.AluOpType.mult,
                    accum_out=red[:, h : h + 1],
                )

            # red[:,3] = sum_j sign(C) — n_pos partials (C >= 0 always)
            nc.scalar.activation(
                scrs, c_sb, mybir.ActivationFunctionType.Sign, accum_out=red[:, 3:4]
            )
            # red[:,4] = sum_j C * S2_i over [896:1024], on DVE
            nc.vector.tensor_scalar(
                scra[:, 896:N],
                c_sb[:, 896:N],
                s2i[:, 0:1],
                0.0,
                op0=mybir.AluOpType.mult,
                op1=mybir.AluOpType.add,
                accum_out=red[:, 4:5],
            )

            # DMA the per-partition partials straight out (2.5KB); the
            # 128-way partition reduce joins the host-side all-reduce.
            # Cheaper than PE-reduce + PSUM->SBUF copy + DMA (~0.5us).
            nc.sync.dma_start(out=o_d[:, :], in_=red)

    _split_multi_waits(nc)
    return nc


def _split_multi_waits(nc: bass.Bass):
    """This walrus build accepts only ONE sync-wait per instruction
    ("Too many sync wait commands"). Tile's kernel-tail drain carries one
    wait per live semaphore; peel the extras onto same-engine NOPs that
    each stall on a single semaphore — semantically identical."""
    for bb in nc.main_func.blocks:
        insts = bb.instructions
        i = 0
        while i < len(insts):
            ins = insts[i]
            si = getattr(ins, "sync_info", None)
            if si is not None and si.on_wait is not None and len(si.on_wait) > 1:
                waits = list(si.on_wait)
                nops = []
                for j, w in enumerate(waits[:-1]):
                    nop = mybir.InstNoOp(
                        name=f"{ins.name}-wsplit{j}",
                        sync_info=mybir.SyncInfo(on_wait=[w], on_update=[]),
                        bass_nofuse=True,
                        engine=ins.engine,
                    )
                    nc.register_instruction(nop, overwrite=True)
                    nops.append(nop)
                si.on_wait = [waits[-1]]
                insts[i:i] = nops
                i += len(nops)
            i += 1


_NC = None


def _get_nc() -> bass.Bass:
    global _NC
    if _NC is None:
        _NC = build_bass()
    return _NC


def make_in_maps(probs: np.ndarray, co_matrix: np.ndarray):
    probs = np.asarray(probs, dtype=np.float32)
    co_matrix = np.asarray(co_matrix, dtype=np.float32)
    in_maps = []
    for k in range(NCORES):
        shift = -SH * k
        p_r = np.roll(probs, shift, axis=1)
        c_r = np.roll(co_matrix[SH * k : SH * (k + 1), :], shift, axis=1)
        paug = np.concatenate([p_r[:, 0:SH].T, p_r], axis=1)
        in_maps.append(
            {
                "paug_r": np.ascontiguousarray(paug.astype(FP8NP)),
                "co_r": np.ascontiguousarray(c_r.astype(FP8NP)),
            }
        )
    return in_maps


def finish(outs: np.ndarray) -> np.ndarray:
    """outs: [NCORES,SH,5] = (stt0, stt1, C*S2_i lo, n_pos, C*S2_i hi)."""
    o = outs.astype(np.float64)
    total = np.float32(o[:, :, 0:3].sum() + o[:, :, 4].sum())
    npos = np.float32(o[:, :, 3].sum())
    loss = (total / np.float32(B)) / (npos + np.float32(1e-8))
    return np.array(loss, dtype=np.float32)


def kernel(probs: np.ndarray, co_matrix: np.ndarray) -> np.ndarray:
    nc = _get_nc()
    in_maps = make_in_maps(probs, co_matrix)
    res = run_bass_kernel_spmd(nc, in_maps, list(range(NCORES)))
    outs = np.stack([r["out"] for r in res.results])
    return finish(outs)


# revision 37
# speedup vs baseline: 1.0093x; 1.0093x over previous
"""Trainium2 Bass kernel for the KB criterion loss.

Math
----
reference:
    diff[b,i,j] = probs[b,j] - probs[b,i]
    loss = sum_ij mean_b (diff^2 * C[i,j]) / (n_pos + 1e-8),  n_pos = count(C > 0)

Expanding the square removes the [B,N,N] intermediate entirely:

    sum_b (P[b,i] - P[b,j])^2 = S2_i + S2_j - 2*G_ij
        with S2_j = sum_b P[b,j]^2   and   G = P^T P  (Gram matrix)

so   total = sum_ij C_ij * (S2_i + S2_j - 2 G_ij)
     loss  = (total / B) / (n_pos + 1e-8)

Sharding (8 cores)
------------------
Shard C by rows: core k owns rows S_k = [128k, 128k+128). P is replicated.
Each core moves ~0.27MB (fp8 [PT|P] + fp8 C shard) vs 4MB of C with
batch-parallel sharding — fewer bytes also shrinks the cross-core HBM
contention window that makes straggler cores. Inputs are column-rolled by 128k so every core
runs the same program with its own row block mapped to local columns
[0:128). The transposed local P block (pure layout prep, like the roll)
rides in front of P in one fused DMA so S2_i is available as a
per-partition column without an on-device transpose.

Per-core pipeline (engines in parallel; inputs fp8e4m3, psq bf16,
accumulation fp32):
  DVE : psq = P*P (256-col quarters, so the S-matmuls start early)
  PE  : per 512-col PSUM bank (one tile per bank so each bank's
        accumulation group closes independently):
           d = P_Sk^T @ P            (Gram block, contract over b=128)
             + (-1/2 const)^T @ psq  (= -S2_j/2 broadcast, 256-col stops)
  ACT : s2i = rowsum(PT_Sk^2)  (Square + accum)  -> S2_i column
  DVE : (C * -2) * d  summed per partition  -> red[:,0:2]  (fused STT)
  ACT : C * s2i [0:896]  summed per partition -> red[:,2] (Copy, scale AP)
  DVE : C * s2i [896:1024]                    -> red[:,4] (tensor_scalar)
  ACT : Sign(C)  summed per partition         -> red[:,3] (n_pos count)
  SP  : DMA the [128,5] partials out; host does the partition reduce.

The ACT/DVE split of the C*S2_i pass balances the two engines' post-C
work. Host sums the 8 partial [128,5] blocks (the scalar all-reduce)
and finishes the division. fp8 input rounding adds a ~0.4% bias on the
squared differences plus random error that cancels over the 2^20
summed terms — observed rel err 5.5e-3 vs the 2e-2 tolerance.
"""

import ml_dtypes
import numpy as np

import concourse.bass as bass
import concourse.tile as tile
from concourse import mybir
from concourse.bass_utils import run_bass_kernel_spmd

B = 128
N = 1024
NCORES = 8
SH = N // NCORES  # 128 rows of C per core
F32 = mybir.dt.float32
BF16 = mybir.dt.bfloat16
HALF = 512  # PSUM bank width in fp32
BF16NP = ml_dtypes.bfloat16
FP8 = mybir.dt.float8e4
FP8NP = ml_dtypes.float8_e4m3


def build_bass() -> bass.Bass:
    nc = bass.Bass(monotonic_sem_count=0, enable_partition_id=False)
    # Drop the dead const-AP memsets (f32-1.0, bf16-1.0, u8-127) from the
    # preamble: they run on Pool, the straggler of the start barrier. The
    # first one (f32-0.0) stays — the Sign bias reads it.
    _mb = nc.main_func.blocks[0]
    _memsets = [i for i in _mb.instructions if isinstance(i, mybir.InstMemset)]
    for _ins in _memsets[1:]:
        _mb.instructions.remove(_ins)

    # paug = [PT_Sk | P] fused into one DMA: one issue slot instead of
    # two, so the C transfer starts ~0.6us earlier.
    paug_d = nc.dram_tensor("paug_r", [B, SH + N], FP8, kind="ExternalInput")
    c_d = nc.dram_tensor("co_r", [SH, N], FP8, kind="ExternalInput")
    o_d = nc.dram_tensor("out", [SH, 5], F32, kind="ExternalOutput")

    with tile.TileContext(nc) as tc:
        with (
            tc.tile_pool(name="sb", bufs=1) as sb,
            tc.tile_pool(name="ps", bufs=1, space="PSUM") as ps,
        ):
            paug_sb = sb.tile([B, SH + N], FP8)
            pt_sb = paug_sb[:, 0:SH]
            p_sb = paug_sb[:, SH : SH + N]
            c_sb = sb.tile([SH, N], FP8)
            psq = sb.tile([B, N], BF16)
            nh = sb.tile([B, SH], BF16)  # const -1/2, lhsT of the S2_j matmul
            ptsq = sb.tile([SH, B], BF16)  # dead store of the Square pass
            s2i = sb.tile([SH, 1], F32)
            scr = sb.tile([SH, N], BF16)  # dead store of the fused STT
            scra = sb.tile([SH, N], BF16)  # dead store of the C*s2i pass
            scrs = sb.tile([SH, N], BF16)  # dead store of the Sign pass
            red = sb.tile([SH, 5], F32)

            # one PSUM tile per bank: a reader of a PSUM tile waits for the
            # tile's whole accumulation group, so separate tiles let the
            # first STT start as soon as bank0's group closes
            d_ps = [
                ps.tile([B, HALF], F32, name=f"d_ps{h}") for h in range(2)
            ]

            # Constant: the consumer S-matmul pairs DVE-produced psq with
            # this, so it is born on DVE to keep matmul waits cheap.
            nc.vector.memset(nh, -0.5)

            # Loads, all on the SP queue: split queues measured slower (the
            # 16 DMA engines are shared and the queues contend). [PT|P]
            # first (heads the compute path), then C.
            nc.sync.dma_start(out=paug_sb, in_=paug_d[:, :])
            nc.sync.dma_start(out=c_sb, in_=c_d[:, :])

            # psq = P*P in quarters so the S-matmuls can start earlier
            Q = 256
            for q in range(4):
                qs = slice(Q * q, Q * (q + 1))
                nc.vector.tensor_mul(psq[:, qs], p_sb[:, qs], p_sb[:, qs])

            # s2i[i] = sum_b PT[i,b]^2 = S2 for the local rows, as a column
            nc.scalar.activation(
                ptsq, pt_sb, mybir.ActivationFunctionType.Square, accum_out=s2i
            )
            # red[:,2] = sum_j C * S2_i over [0:896] (scale is a
            # per-partition column); the last eighth runs on DVE (red[:,4])
            # to balance the two engines' post-C work
            nc.scalar.activation(
                scra[:, 0:896],
                c_sb[:, 0:896],
                mybir.ActivationFunctionType.Copy,
                scale=s2i[:, 0:1],
                accum_out=red[:, 2:3],
            )

            # d = G - S2_j/2 accumulated per PSUM bank; the S term lands in
            # 256-col quarters so each bank's tail quarter stops sooner
            for h in range(2):
                js = slice(HALF * h, HALF * (h + 1))
                nc.tensor.matmul(
                    d_ps[h][:, :], p_sb[:, 0:SH], p_sb[:, js], start=True, stop=False
                )
                for hq in range(2):
                    qs = slice(Q * hq, Q * (hq + 1))
                    nc.tensor.matmul(
                        d_ps[h][:, qs],
                        nh,
                        psq[:, HALF * h + Q * hq : HALF * h + Q * (hq + 1)],
                        start=False,
                        stop=True,
                    )

            # red[:,h] = sum_j (C * -2) * d  (fused multiply+accumulate)
            for h in range(2):
                js = slice(HALF * h, HALF * (h + 1))
                nc.vector.scalar_tensor_tensor(
                    scr[:, js],
                    c_sb[:, js],
                    -2.0,
                    d_ps[h][:, :],
                    op0=mybir.AluOpType.mult,
                    op1=mybir.AluOpType.mult,
                    accum_out=red[:, h : h + 1],
                )

            # red[:,3] = sum_j sign(C) — n_pos partials (C >= 0 always)
            nc.scalar.activation(
                scrs, c_sb, mybir.ActivationFunctionType.Sign, accum_out=red[:, 3:4]
            )
            # red[:,4] = sum_j C * S2_i over [896:1024], on DVE
            nc.vector.tensor_scalar(
                scra[:, 896:N],
                c_sb[:, 896:N],
                s2i[:, 0:1],
                0.0,
                op0=mybir.AluOpType.mult,
                op1=mybir.AluOpType.add,
                accum_out=red[:, 4:5],
            )

            # DMA the per-partition partials straight out (2.5KB); the
            # 128-way partition reduce joins the host-side all-reduce.
            # Cheaper than PE-reduce + PSUM->SBUF copy + DMA (~0.5us).
            nc.sync.dma_start(out=o_d[:, :], in_=red)

    _split_multi_waits(nc)
    return nc


def _split_multi_waits(nc: bass.Bass):
    """This walrus build accepts only ONE sync-wait per instruction
    ("Too many sync wait commands"). Tile's kernel-tail drain carries one
    wait per live semaphore; peel the extras onto same-engine NOPs that
    each stall on a single semaphore — semantically identical."""
    for bb in nc.main_func.blocks:
        insts = bb.instructions
        i = 0
        while i < len(insts):
            ins = insts[i]
            si = getattr(ins, "sync_info", None)
            if si is not None and si.on_wait is not None and len(si.on_wait) > 1:
                waits = list(si.on_wait)
                nops = []
                for j, w in enumerate(waits[:-1]):
                    nop = mybir.InstNoOp(
                        name=f"{ins.name}-wsplit{j}",
                        sync_info=mybir.SyncInfo(on_wait=[w], on_update=[]),
                        bass_nofuse=True,
                        engine=ins.engine,
                    )
                    nc.register_instruction(nop, overwrite=True)
                    nops.append(nop)
                si.on_wait = [waits[-1]]
                insts[i:i] = nops
                i += len(nops)
            i += 1


_NC = None


def _get_nc() -> bass.Bass:
    global _NC
    if _NC is None:
        _NC = build_bass()
    return _NC


def make_in_maps(probs: np.ndarray, co_matrix: np.ndarray):
    probs = np.asarray(probs, dtype=np.float32)
    co_matrix = np.asarray(co_matrix, dtype=np.float32)
    in_maps = []
    for k in range(NCORES):
        shift = -SH * k
        p_r = np.roll(probs, shift, axis=1)
        c_r = np.roll(co_matrix[SH * k : SH * (k + 1), :], shift, axis=1)
        paug = np.concatenate([p_r[:, 0:SH].T, p_r], axis=1)
        in_maps.append(
            {
                "paug_r": np.ascontiguousarray(paug.astype(FP8NP)),
                "co_r": np.ascontiguousarray(c_r.astype(FP8NP)),
            }
        )
    return in_maps


def finish(outs: np.ndarray) -> np.ndarray:
    """outs: [NCORES,SH,5] = (stt0, stt1, C*S2_i lo, n_pos, C*S2_i hi)."""
    o = outs.astype(np.float64)
    total = np.float32(o[:, :, 0:3].sum() + o[:, :, 4].sum())
    npos = np.float32(o[:, :, 3].sum())
    loss = (total / np.float32(B)) / (npos + np.float32(1e-8))
    return np.array(loss, dtype=np.float32)


def kernel(probs: np.ndarray, co_matrix: np.ndarray) -> np.ndarray:
    nc = _get_nc()
    in_maps = make_in_maps(probs, co_matrix)
    res = run_bass_kernel_spmd(nc, in_maps, list(range(NCORES)))
    outs = np.stack([r["out"] for r in res.results])
    return finish(outs)


# revision 40
# speedup vs baseline: 1.0402x; 1.0306x over previous
"""Trainium2 Bass kernel for the KB criterion loss.

Math
----
reference:
    diff[b,i,j] = probs[b,j] - probs[b,i]
    loss = sum_ij mean_b (diff^2 * C[i,j]) / (n_pos + 1e-8),  n_pos = count(C > 0)

Expanding the square removes the [B,N,N] intermediate entirely:

    sum_b (P[b,i] - P[b,j])^2 = S2_i + S2_j - 2*G_ij
        with S2_j = sum_b P[b,j]^2   and   G = P^T P  (Gram matrix)

so   total = sum_ij C_ij * (S2_i + S2_j - 2 G_ij)
     loss  = (total / B) / (n_pos + 1e-8)

Sharding (8 cores)
------------------
Shard C by rows: core k owns rows S_k = [128k, 128k+128). P is replicated.
Each core moves ~0.27MB (fp8 [PT|P] + fp8 C shard) vs 4MB of C with
batch-parallel sharding — fewer bytes also shrinks the cross-core HBM
contention window that makes straggler cores. Inputs are column-rolled by 128k so every core
runs the same program with its own row block mapped to local columns
[0:128). The transposed local P block (pure layout prep, like the roll)
rides in front of P in one fused DMA so S2_i is available as a
per-partition column without an on-device transpose.

Per-core pipeline (engines in parallel; inputs fp8e4m3, psq bf16,
accumulation fp32):
  DVE : psq = P*P (256-col quarters, so the S-matmuls start early)
  PE  : per 512-col PSUM bank (one tile per bank so each bank's
        accumulation group closes independently):
           d = P_Sk^T @ P            (Gram block, contract over b=128)
             + (-1/2 const)^T @ psq  (= -S2_j/2 broadcast, 256-col stops)
  ACT : s2i = rowsum(PT_Sk^2)  (Square + accum)  -> S2_i column
  DVE : (C * -2) * d  summed per partition  -> red[:,0:2]  (fused STT)
  ACT : C * s2i [0:896]  summed per partition -> red[:,2] (Copy, scale AP)
  DVE : C * s2i [896:1024]                    -> red[:,4] (tensor_scalar)
  ACT : Sign(C)  summed per partition         -> red[:,3] (n_pos count)
  SP  : DMA the [128,5] partials out; host does the partition reduce.

The ACT/DVE split of the C*S2_i pass balances the two engines' post-C
work. Host sums the 8 partial [128,5] blocks (the scalar all-reduce)
and finishes the division. fp8 input rounding adds a ~0.4% bias on the
squared differences plus random error that cancels over the 2^20
summed terms — observed rel err 5.5e-3 vs the 2e-2 tolerance.
"""

import ml_dtypes
import numpy as np

import concourse.bass as bass
import concourse.tile as tile
from concourse import mybir
from concourse.bass_utils import run_bass_kernel_spmd

B = 128
N = 1024
NCORES = 8
SH = N // NCORES  # 128 rows of C per core
F32 = mybir.dt.float32
BF16 = mybir.dt.bfloat16
HALF = 512  # PSUM bank width in fp32
BF16NP = ml_dtypes.bfloat16
FP8 = mybir.dt.float8e4
FP8NP = ml_dtypes.float8_e4m3


def build_bass() -> bass.Bass:
    nc = bass.Bass(monotonic_sem_count=0, enable_partition_id=False)
    # Drop the dead const-AP memsets (f32-1.0, bf16-1.0, u8-127) from the
    # preamble: they run on Pool, the straggler of the start barrier. The
    # first one (f32-0.0) stays — the Sign bias reads it.
    _mb = nc.main_func.blocks[0]
    _memsets = [i for i in _mb.instructions if isinstance(i, mybir.InstMemset)]
    for _ins in _memsets[1:]:
        _mb.instructions.remove(_ins)

    # paug = [PT_Sk | P] fused into one DMA: one issue slot instead of
    # two, so the C transfer starts ~0.6us earlier.
    paug_d = nc.dram_tensor("paug_r", [B, SH + N], FP8, kind="ExternalInput")
    c_d = nc.dram_tensor("co_r", [SH, N], FP8, kind="ExternalInput")
    o_d = nc.dram_tensor("out", [SH, 5], F32, kind="ExternalOutput")

    with tile.TileContext(nc) as tc:
        with (
            tc.tile_pool(name="sb", bufs=1) as sb,
            tc.tile_pool(name="ps", bufs=1, space="PSUM") as ps,
        ):
            paug_sb = sb.tile([B, SH + N], FP8)
            pt_sb = paug_sb[:, 0:SH]
            p_sb = paug_sb[:, SH : SH + N]
            c_sb = sb.tile([SH, N], FP8)
            psq = sb.tile([B, N], BF16)
            nh = sb.tile([B, SH], BF16)  # const -1/2, lhsT of the S2_j matmul
            ptsq = sb.tile([SH, B], BF16)  # dead store of the Square pass
            s2i = sb.tile([SH, 1], F32)
            scr = sb.tile([SH, N], BF16)  # dead store of the fused STT
            scra = sb.tile([SH, N], BF16)  # dead store of the C*s2i pass
            scrs = sb.tile([SH, N], BF16)  # dead store of the Sign pass
            red = sb.tile([SH, 5], F32)

            # one PSUM tile per bank: a reader of a PSUM tile waits for the
            # tile's whole accumulation group, so separate tiles let the
            # first STT start as soon as bank0's group closes
            d_ps = [
                ps.tile([B, HALF], F32, name=f"d_ps{h}") for h in range(2)
            ]

            # Constant: the consumer S-matmul pairs DVE-produced psq with
            # this, so it is born on DVE to keep matmul waits cheap.
            nc.vector.memset(nh, -0.5)

            # Loads, all on the SP queue: split queues measured slower (the
            # 16 DMA engines are shared and the queues contend). [PT|P]
            # first (heads the compute path), then C.
            nc.sync.dma_start(out=paug_sb, in_=paug_d[:, :])
            nc.sync.dma_start(out=c_sb, in_=c_d[:, :])

            # psq = P*P in quarters so the S-matmuls can start earlier
            Q = 256
            for q in range(4):
                qs = slice(Q * q, Q * (q + 1))
                nc.vector.tensor_mul(psq[:, qs], p_sb[:, qs], p_sb[:, qs])

            # s2i[i] = sum_b PT[i,b]^2 = S2 for the local rows, as a column
            nc.scalar.activation(
                ptsq, pt_sb, mybir.ActivationFunctionType.Square, accum_out=s2i
            )
            # red[:,2] = sum_j C * S2_i over [0:896] (scale is a
            # per-partition column); the last eighth runs on DVE (red[:,4])
            # to balance the two engines' post-C work
            nc.scalar.activation(
                scra[:, 0:896],
                c_sb[:, 0:896],
                mybir.ActivationFunctionType.Copy,
                scale=s2i[:, 0:1],
                accum_out=red[:, 2:3],
            )

            # d = G - S2_j/2 accumulated per PSUM bank; the S term lands in
            # 256-col quarters so each bank's tail quarter stops sooner
            for h in range(2):
                js = slice(HALF * h, HALF * (h + 1))
                nc.tensor.matmul(
                    d_ps[h][:, :], p_sb[:, 0:SH], p_sb[:, js], start=True, stop=False
                )
                for hq in range(2):
                    qs = slice(Q * hq, Q * (hq + 1))
                    nc.tensor.matmul(
                        d_ps[h][:, qs],
                        nh,
                        psq[:, HALF * h + Q * hq : HALF * h + Q * (hq + 1)],
                        start=False,
                        stop=True,
                    )

            # red[:,h] = sum_j (C * -2) * d  (fused multiply+accumulate)
            for h in range(2):
                js = slice(HALF * h, HALF * (h + 1))
                nc.vector.scalar_tensor_tensor(
                    scr[:, js],
                    c_sb[:, js],
                    -2.0,
                    d_ps[h][:, :],
                    op0=mybir.AluOpType.mult,
                    op1=mybir.AluOpType.mult,
                    accum_out=red[:, h : h + 1],
                )

            # red[:,3] = sum_j sign(C) — n_pos partials (C >= 0 always)
            nc.scalar.activation(
                scrs, c_sb, mybir.ActivationFunctionType.Sign, accum_out=red[:, 3:4]
            )
            # red[:,4] = sum_j C * S2_i over [896:1024], on DVE
            nc.vector.tensor_scalar(
                scra[:, 896:N],
                c_sb[:, 896:N],
                s2i[:, 0:1],
                0.0,
                op0=mybir.AluOpType.mult,
                op1=mybir.AluOpType.add,
                accum_out=red[:, 4:5],
            )

            # DMA the per-partition partials straight out (2.5KB); the
            # 128-way partition reduce joins the host-side all-reduce.
            # Cheaper than PE-reduce + PSUM->SBUF copy + DMA (~0.5us).
            nc.sync.dma_start(out=o_d[:, :], in_=red)

    _split_multi_waits(nc)
    return nc


def _split_multi_waits(nc: bass.Bass):
    """This walrus build accepts only ONE sync-wait per instruction
    ("Too many sync wait commands"). Tile's kernel-tail drain carries one
    wait per live semaphore; peel the extras onto same-engine NOPs that
    each stall on a single semaphore — semantically identical."""
    for bb in nc.main_func.blocks:
        insts = bb.instructions
        i = 0
        while i < len(insts):
            ins = insts[i]
            si = getattr(ins, "sync_info", None)
            if si is not None and si.on_wait is not None and len(si.on_wait) > 1:
                waits = list(si.on_wait)
                nops = []
                for j, w in enumerate(waits[:-1]):
                    nop = mybir.InstNoOp(
                        name=f"{ins.name}-wsplit{j}",
                        sync_info=mybir.SyncInfo(on_wait=[w], on_update=[]),
                        bass_nofuse=True,
                        engine=ins.engine,
                    )
                    nc.register_instruction(nop, overwrite=True)
                    nops.append(nop)
                si.on_wait = [waits[-1]]
                insts[i:i] = nops
                i += len(nops)
            i += 1


_NC = None


def _get_nc() -> bass.Bass:
    global _NC
    if _NC is None:
        _NC = build_bass()
    return _NC


def make_in_maps(probs: np.ndarray, co_matrix: np.ndarray):
    probs = np.asarray(probs, dtype=np.float32)
    co_matrix = np.asarray(co_matrix, dtype=np.float32)
    in_maps = []
    for k in range(NCORES):
        shift = -SH * k
        p_r = np.roll(probs, shift, axis=1)
        c_r = np.roll(co_matrix[SH * k : SH * (k + 1), :], shift, axis=1)
        paug = np.concatenate([p_r[:, 0:SH].T, p_r], axis=1)
        in_maps.append(
            {
                "paug_r": np.ascontiguousarray(paug.astype(FP8NP)),
                "co_r": np.ascontiguousarray(c_r.astype(FP8NP)),
            }
        )
    return in_maps


def finish(outs: np.ndarray) -> np.ndarray:
    """outs: [NCORES,SH,5] = (stt0, stt1, C*S2_i lo, n_pos, C*S2_i hi)."""
    o = outs.astype(np.float64)
    total = np.float32(o[:, :, 0:3].sum() + o[:, :, 4].sum())
    npos = np.float32(o[:, :, 3].sum())
    loss = (total / np.float32(B)) / (npos + np.float32(1e-8))
    return np.array(loss, dtype=np.float32)


def kernel(probs: np.ndarray, co_matrix: np.ndarray) -> np.ndarray:
    nc = _get_nc()
    in_maps = make_in_maps(probs, co_matrix)
    res = run_bass_kernel_spmd(nc, in_maps, list(range(NCORES)))
    outs = np.stack([r["out"] for r in res.results])
    return finish(outs)
